# revision 5
# baseline (speedup 1.0000x reference)
"""DeepTensorNeuralNetwork (DTNN / gnn_message_passing) Trainium2 kernel.

Math (per reference):
    d_sum = distance.sum(axis=2)                                  # (B,N,R)
    for l in 0..2:
        cf = x @ Wcf[l].T + bcf[l]                                # (B,N,H)
        df = d_sum @ Wdf[l].T + N*bdf[l]                          # (B,N,H)
        h  = (cf*df) @ Wfc[l].T                                   # (B,N,F)
        x  = h + tanh(h)
    g = x.sum(axis=1); out = (g @ fc0.T + b0) @ ow.T + ob         # (B,1)

Strategy: data-parallel over batch across 8 NeuronCores (8 batches each).
The dominant cost is streaming `distance` (33.5 MB/core) from HBM; the
16 hardware DMA queues sustain ~26 GB/s each (~420 GB/s aggregate) when
fed.  Two structural lessons from the previous version's trace drive
this layout:
  * An HWDGE ring holds only 4 outstanding transfers, and a dma_start
    on a full ring BLOCKS the issuing engine's whole in-order queue.
    The scalar (ACT) engine once stalled 24us on a push, freezing every
    bias/tanh behind it.  So: ALL steady-state distance pushes go on
    the sync engine (which does nothing else and may stall harmlessly);
    the scalar ring gets only the weight pack + batch 0 (5 pushes, at
    most a ~1us stall at t~4us before any ACT work exists).
  * 4096-col chunks produced 16KB descriptors on hw queue 15 vs 8KB on
    queues 0-14 (equal counts), making queue 15 a 2x-loaded straggler
    that ran alone for the last ~13us of the stream.  2048-col chunks
    (8KB rows) make every descriptor 8KB so the queues stay balanced.
The j-reduction is a DVE binary fold tree per 1MB chunk (first fold
casts fp32->fp16 so later folds run in the DVE 2x packed mode), then 3
combine adds per batch.  A chunk tile is fully read by its first fold
op, so buffers recycle fast (bufs=8 = 2 batches in flight).  Layer
compute runs in batch groups (4,2,1,1): matmuls on PE, bias/tanh on
ACT, elementwise on DVE, software-pipelined so the in-order engine
queues reach every op with deps already met.  The final 1-batch group
is the only work after the last chunk lands: its cf0 is hoisted early,
its df for all 3 layers is computed right after its d_sum transpose,
and filler ops from earlier groups hide the cross-engine latency of
its 3-layer chain.  The affine head (fc0 + out) is folded on the host
into a single length-F vector + scalar bias.  All constants ship in
ONE packed array -> one DMA -> one wait sem.
"""

import numpy as np

B, N, F, R, H = 64, 128, 128, 64, 256
L = 3
NCORES = 8
BL = B // NCORES   # batches per core
CW = 2048          # dist chunk width in fp32 cols (8KB rows -> even descriptors)
NCHUNK = (N * R) // CW  # 4 chunks per batch
GROUPS = ((0, 1, 2, 3), (4, 5), (6,), (7,))

# wpack layout, fp32 columns (fp16 sections hold 2 fp16 per column;
# offsets in the comments below are fp16-element columns of the bitcast view):
#   [0, 384)      wcf lhsT bf16 : bf-col l*H+h           = Wcf_w[l, h, f]
#   [384, 768)    wfc lhsT bf16 : bf-col (l*2+c)*F+f     = Wfc_w[l, f, c*128+hc]
#   [768, 774)    cf bias fp32  : col l*2+c              = Wcf_b[l, c*128+h]
#   [774, 780)    df bias fp32  : col l*2+c              = N * Wdf_b[l, c*128+h]
#   [780, 781)    head lhsT bf16: bf-col 0               = (out_w @ fc0_w)[0, f]
#   [784, 1168)   wdf lhsT bf16 : bf-col l*H+h (rows<64) = Wdf_w[l, h, r]
#   [1168, 1680)  x bf16        : bf-col b*N+n           = x[b_local, n, f]
#   [1680, 1744)  identity fp16-packed
BCF_OFF = 768
BDF_OFF = 774
HEAD_OFF = 780
HEAD32_OFF = 781
WDF_OFF = 784
XOFF = 1168
IDOFF = 1680
WCOLS = 1808

_CACHE = {}


def _build_program():
    import concourse.bass as bass
    from concourse import bacc
    import concourse.tile as tile
    from concourse import mybir

    f32 = mybir.dt.float32
    bf16 = mybir.dt.float16
    AX = mybir.AxisListType
    AF = mybir.ActivationFunctionType

    nc = bacc.Bacc("TRN2")
    dist = nc.declare_dram_parameter("dist", [BL, N, N, R], f32, isOutput=False)
    wpack = nc.declare_dram_parameter("wpack", [128, WCOLS], f32, isOutput=False)
    out_ext = nc.declare_dram_parameter("out", [BL, 1], f32, isOutput=True)

    with tile.TileContext(nc) as tc:
        with (
            tc.tile_pool(name="consts", bufs=1) as consts,
            tc.tile_pool(name="dist", bufs=8) as dist_pool,
            tc.tile_pool(name="fold", bufs=2) as fold_pool,
            tc.tile_pool(name="dsum", bufs=2) as dsum_pool,
            tc.tile_pool(name="work", bufs=3) as work,
            tc.tile_pool(name="ps1", bufs=1, space="PSUM") as ps1,
            tc.tile_pool(name="ps2", bufs=2, space="PSUM") as ps2,
        ):
            # ---- DMA plumbing -------------------------------------------
            # weight pack: the scalar ring's first push (never stalls ACT)
            wp = consts.tile([128, WCOLS], f32)
            nc.scalar.dma_start(out=wp, in_=wpack[:, :])
            wb = wp.bitcast(bf16)  # (128, 2*WCOLS) bf16 view
            ident = wb[:, 2 * IDOFF : 2 * IDOFF + 128]
            out_acc = consts.tile([1, BL], f32)

            chunk_order = [(b, q) for b in range(BL) for q in range(NCHUNK)]
            chunk_tiles = {}
            push_cursor = [0]

            def push_chunk():
                if push_cursor[0] >= len(chunk_order):
                    return
                b, q = chunk_order[push_cursor[0]]
                push_cursor[0] += 1
                t = dist_pool.tile([N, CW], f32, tag="dist")
                dflat = dist[b, :, :, :].rearrange("n j r -> n (j r)")
                eng = nc.scalar if b == 0 else nc.sync
                eng.dma_start(out=t, in_=dflat[:, q * CW : (q + 1) * CW])
                chunk_tiles[(b, q)] = t

            for _ in range(2 * NCHUNK):  # prime: batches 0 (scalar) + 1 (sync)
                push_chunk()

            def wcf_l(l, c):
                o = l * H + c * 128
                return wb[:, o : o + 128]

            def wdf_l(l, c):
                o = 2 * WDF_OFF + l * H + c * 128
                return wb[0:R, o : o + 128]

            def wfc_l(l, c):
                o = 2 * 384 + (l * 2 + c) * F
                return wb[:, o : o + F]

            def bcf_l(l, c):
                o = BCF_OFF + l * 2 + c
                return wp[:, o : o + 1]

            def bdf_l(l, c):
                o = BDF_OFF + l * 2 + c
                return wp[:, o : o + 1]

            # ---- j-reduction: per-chunk DVE fold trees ------------------
            def fold_chunk(b, q, final_out):
                """DVE fold tree over one (N, CW) fp32 chunk -> 64 fp16
                cols into final_out.  First fold casts fp32->fp16; later
                folds run packed.  The first op reads the whole chunk, so
                the tile frees immediately -> push the next chunk then."""
                src = chunk_tiles.pop((b, q))
                hw = CW // 2
                s = fold_pool.tile([N, hw], bf16, tag="s", name="s")
                nc.vector.tensor_add(s[:, 0:hw], src[:, 0:hw], src[:, hw:CW])
                push_chunk()
                t = fold_pool.tile([N, hw // 2], bf16, tag="t", name="t")
                cur, other, w = s, t, hw // 2
                while w >= 64:
                    dst = final_out if w == 64 else other[:, 0:w]
                    nc.vector.tensor_add(dst, cur[:, 0:w], cur[:, w : 2 * w])
                    cur, other = other, cur
                    w //= 2

            dsums = {}

            def fold_batch(b, tail=0):
                """Fold chunks 0..NCHUNK-1-tail of batch b (tail>0 lets the
                caller interleave other work before the last chunks)."""
                quad = dsum_pool.tile([N, NCHUNK * 64], bf16, tag="quad",
                                      name=f"quad{b}")
                dsums[b] = ("quad", quad)
                for q in range(NCHUNK - tail):
                    fold_chunk(b, q, quad[:, q * 64 : (q + 1) * 64])

            def fold_finish(b, tail):
                _, quad = dsums[b]
                for q in range(NCHUNK - tail, NCHUNK):
                    fold_chunk(b, q, quad[:, q * 64 : (q + 1) * 64])
                h0 = dsum_pool.tile([N, R], bf16, tag="h", name=f"h0_{b}")
                nc.vector.tensor_add(h0, quad[:, 0:64], quad[:, 64:128])
                h1 = dsum_pool.tile([N, R], bf16, tag="h", name=f"h1_{b}")
                nc.vector.tensor_add(h1, quad[:, 128:192], quad[:, 192:256])
                dsum = dsum_pool.tile([N, R], bf16, tag="dsum", name=f"ds{b}")
                nc.vector.tensor_add(dsum, h0, h1)
                dsums[b] = ("dsum", dsum)

            # ---- layer pipeline (per batch group) -----------------------
            gstate = {}

            def emit_trs(gi):
                bs = GROUPS[gi]
                dsT = dsum_pool.tile([R, 4 * N], bf16, tag="dsT",
                                     name=f"dsT{gi}")
                for k, b in enumerate(bs):
                    kind, ds = dsums.pop(b)
                    assert kind == "dsum"
                    trp = ps1.tile([R, N], bf16, tag="tr")
                    nc.tensor.transpose(trp, ds, ident)
                    nc.scalar.activation(
                        out=dsT[:, k * N : (k + 1) * N], in_=trp, func=AF.Copy
                    )
                NG = len(bs) * N
                xc = wb[:, 2 * XOFF + bs[0] * N : 2 * XOFF + (bs[-1] + 1) * N]
                gstate.setdefault(gi, {}).update(
                    {"dsT": dsT, "xc": xc, "NG": NG, "bs": bs}
                )

            def emit_cf_hoist(gi):
                # layer-0 cf depends only on the x pack: compute any time
                bs = GROUPS[gi]
                NG = len(bs) * N
                xc = wb[:, 2 * XOFF + bs[0] * N : 2 * XOFF + (bs[-1] + 1) * N]
                res = []
                for c in range(2):
                    cfp = ps1.tile([128, 4 * N], f32, tag=f"cf{c}",
                                   name=f"cfp{c}")[:, 0:NG]
                    nc.tensor.matmul(cfp, wcf_l(0, c), xc, start=True, stop=True)
                    cfs = work.tile([128, N], bf16, tag=f"cfH{gi}{c}",
                                    name=f"cfH{gi}{c}", bufs=1)[:, 0:NG]
                    nc.scalar.activation(out=cfs, in_=cfp, func=AF.Identity,
                                         bias=bcf_l(0, c))
                    res.append(cfs)
                gstate.setdefault(gi, {})["cfs0"] = res

            def emit_cf(gi, l):
                st = gstate[gi]
                NG, xc = st["NG"], st["xc"]
                res = []
                for c in range(2):
                    cfp = ps1.tile([128, 4 * N], f32, tag=f"cf{c}",
                                   name=f"cfp{c}")[:, 0:NG]
                    nc.tensor.matmul(cfp, wcf_l(l, c), xc, start=True, stop=True)
                    cfs = work.tile([128, 4 * N], bf16, tag=f"cfs{c}",
                                    name=f"cfs{c}")[:, 0:NG]
                    nc.scalar.activation(out=cfs, in_=cfp, func=AF.Identity,
                                         bias=bcf_l(l, c))
                    res.append(cfs)
                return res

            def emit_df_hoist(gi):
                # df for every layer depends only on dsT: compute it all
                # right after the transpose (shrinks the tail group's
                # post-fold critical chain to cf/mul/fc/tanh/add only)
                st = gstate[gi]
                NG, dsT = st["NG"], st["dsT"]
                st["dfs"] = {}
                for l in range(L):
                    for c in range(2):
                        dfp = ps1.tile([128, 4 * N], f32, tag=f"df{c}",
                                       name=f"dfp{c}")[:, 0:NG]
                        nc.tensor.matmul(dfp, wdf_l(l, c), dsT[:, 0:NG],
                                         start=True, stop=True)
                        dfs = work.tile([128, N], bf16, tag=f"dfsP{c}{l}",
                                        name=f"dfsP{c}{l}", bufs=1)[:, 0:NG]
                        nc.scalar.activation(out=dfs, in_=dfp, func=AF.Identity,
                                             bias=bdf_l(l, c))
                        st["dfs"][(l, c)] = dfs

            def emit_layer(gi, l):
                st = gstate[gi]
                NG, dsT = st["NG"], st["dsT"]
                pre_dfs = st.get("dfs")
                pre_cfs = st.pop("cfs0", None) if l == 0 else None
                if pre_cfs is None:
                    pre_cfs = emit_cf(gi, l)
                ms = []
                for c in range(2):
                    cfs = pre_cfs[c]
                    if pre_dfs is not None:
                        dfs = pre_dfs.pop((l, c))
                    else:
                        dfp = ps1.tile([128, 4 * N], f32, tag=f"df{c}",
                                       name=f"dfp{c}")[:, 0:NG]
                        nc.tensor.matmul(dfp, wdf_l(l, c), dsT[:, 0:NG],
                                         start=True, stop=True)
                        dfs = work.tile([128, 4 * N], bf16, tag=f"dfs{c}",
                                        name=f"dfs{c}")[:, 0:NG]
                        nc.scalar.activation(out=dfs, in_=dfp, func=AF.Identity,
                                             bias=bdf_l(l, c))
                    m = work.tile([128, 4 * N], bf16, tag=f"m{c}",
                                  name=f"m{c}")[:, 0:NG]
                    nc.vector.tensor_mul(m, cfs, dfs)
                    ms.append(m)
                hp = ps2.tile([F, 4 * N], f32, tag="h", name="hp")[:, 0:NG]
                nc.tensor.matmul(hp, wfc_l(l, 0), ms[0], start=True, stop=False)
                nc.tensor.matmul(hp, wfc_l(l, 1), ms[1], start=False, stop=True)
                th = work.tile([F, 4 * N], f32, tag="t", name="th")[:, 0:NG]
                nc.scalar.activation(out=th, in_=hp, func=AF.Tanh)
                xdt = f32 if l == L - 1 else bf16
                xn = work.tile([F, 4 * N], xdt, tag=f"x{l}", name="xn")[:, 0:NG]
                nc.vector.tensor_add(xn, hp, th)
                st["xc"] = xn

            def emit_head(gi):
                st = gstate[gi]
                NG, bs = st["NG"], st["bs"]
                G = len(bs)
                hd = ps1.tile([1, 4 * N], f32, tag="hd", name="hd")[:, 0:NG]
                nc.tensor.matmul(hd, wp[:, HEAD32_OFF : HEAD32_OFF + 1],
                                 st["xc"], start=True, stop=True)
                nc.vector.tensor_reduce(
                    out=out_acc[0:1, bs[0] : bs[0] + G],
                    in_=hd.rearrange("o (b n) -> o b n", b=G),
                    axis=AX.X,
                    op=mybir.AluOpType.add,
                )

            # ---- software-pipelined schedule ----------------------------
            # In-order engine queues: folds must run as data lands (never
            # behind layer ops whose deps aren't ready), and the tail
            # group's chain must start immediately after the last fold,
            # with earlier groups' leftovers as latency-hiding filler.
            for b in (0, 1, 2, 3):
                fold_batch(b)
                fold_finish(b, 0)
            emit_trs(0)
            emit_cf_hoist(2)
            emit_cf_hoist(3)
            fold_batch(4)
            fold_finish(4, 0)
            emit_layer(0, 0)
            fold_batch(5)
            fold_finish(5, 0)
            emit_layer(0, 1)
            emit_trs(1)
            emit_layer(0, 2)
            emit_head(0)
            fold_batch(6)
            fold_finish(6, 0)
            emit_layer(1, 0)
            emit_trs(2)
            emit_layer(1, 1)
            fold_batch(7, tail=1)   # chunks 0-2 of the last batch
            emit_layer(2, 0)
            fold_finish(7, tail=1)  # last chunk + combines: ends the stream
            emit_layer(1, 2)        # DVE filler while trs/df of the tail
            emit_head(1)            # group go through PE/ACT
            emit_trs(3)
            emit_df_hoist(3)
            emit_layer(3, 0)
            emit_layer(2, 1)
            emit_layer(3, 1)
            emit_layer(2, 2)
            emit_head(2)
            emit_layer(3, 2)
            emit_head(3)

            nc.sync.dma_start(out=out_ext.rearrange("b o -> o b"), in_=out_acc)

    return nc


def _host_pack(x, Wcf_w, Wcf_b, Wdf_w, Wdf_b, Wfc_w, fc0_w, fc0_b, out_w, out_b):
    import ml_dtypes

    f = np.float32
    bf = np.float16

    def pack_bf(a):  # (128, 2K) bf16 -> (128, K) fp32 bit-packed
        return np.ascontiguousarray(a.astype(bf)).view(f)

    base = np.zeros((128, WCOLS), f)
    base[:, 0:384] = pack_bf(np.asarray(Wcf_w, f).transpose(2, 0, 1).reshape(128, L * H))
    base[:, 384:768] = pack_bf(
        np.asarray(Wfc_w, f).reshape(L, F, 2, 128).transpose(3, 0, 2, 1).reshape(128, L * 2 * F)
    )
    base[:, BCF_OFF : BCF_OFF + 6] = (
        np.asarray(Wcf_b, f).reshape(L, 2, 128).transpose(2, 0, 1).reshape(128, 6)
    )
    base[:, BDF_OFF : BDF_OFF + 6] = (
        (N * np.asarray(Wdf_b, f)).reshape(L, 2, 128).transpose(2, 0, 1).reshape(128, 6)
    )
    w_head = (np.asarray(out_w, np.float64) @ np.asarray(fc0_w, np.float64))[0]  # (F,)
    head_pair = np.zeros((128, 2), f)
    head_pair[:, 0] = w_head.astype(f)
    base[:, HEAD_OFF : HEAD_OFF + 1] = pack_bf(head_pair)
    base[:, HEAD32_OFF] = w_head.astype(f)
    base[0:R, WDF_OFF : WDF_OFF + 384] = pack_bf(
        np.asarray(Wdf_w, f).transpose(2, 0, 1).reshape(R, L * H)
    )
    base[:, IDOFF : IDOFF + 64] = pack_bf(np.eye(128, dtype=f))

    b_head = float((np.asarray(out_w, np.float64) @ np.asarray(fc0_b, np.float64)
                    + np.asarray(out_b, np.float64)).reshape(()))

    x_t = np.asarray(x, f).transpose(0, 2, 1)  # (B, F, N)
    wpacks = []
    for i in range(NCORES):
        wp = base.copy()
        wp[:, XOFF : XOFF + BL * N // 2] = pack_bf(
            x_t[i * BL : (i + 1) * BL].transpose(1, 0, 2).reshape(128, BL * N)
        )
        wpacks.append(wp)
    return wpacks, b_head


def run(trace=False, **inputs):
    from concourse.bass_utils import run_bass_kernel_spmd

    distance = np.ascontiguousarray(np.asarray(inputs["distance"], np.float32))
    wpacks, b_head = _host_pack(
        inputs["x"], inputs["Wcf_w"], inputs["Wcf_b"], inputs["Wdf_w"], inputs["Wdf_b"],
        inputs["Wfc_w"], inputs["fc0_w"], inputs["fc0_b"], inputs["out_w"], inputs["out_b"],
    )

    if "nc" not in _CACHE:
        nc = _build_program()
        nc.finalize()
        _CACHE["nc"] = nc
    nc = _CACHE["nc"]

    in_maps = []
    for i in range(NCORES):
        in_maps.append({
            "dist": np.ascontiguousarray(distance[i * BL : (i + 1) * BL]),
            "wpack": wpacks[i],
        })
    res = run_bass_kernel_spmd(nc, in_maps, list(range(NCORES)), trace=trace)
    out = np.concatenate([res.results[i]["out"] for i in range(NCORES)], axis=0)
    out = (out.astype(np.float64) + b_head).astype(np.float32)
    return out, res


def kernel(**inputs):
    out, _ = run(trace=False, **inputs)
    return out


# revision 20
# speedup vs baseline: 1.3146x; 1.3146x over previous
"""DeepTensorNeuralNetwork (DTNN / gnn_message_passing) Trainium2 kernel.

Math (per reference):
    d_sum = distance.sum(axis=2)                                  # (B,N,R)
    for l in 0..2:
        cf = x @ Wcf[l].T + bcf[l]                                # (B,N,H)
        df = d_sum @ Wdf[l].T + N*bdf[l]                          # (B,N,H)
        h  = (cf*df) @ Wfc[l].T                                   # (B,N,F)
        x  = h + tanh(h)
    g = x.sum(axis=1); out = (g @ fc0.T + b0) @ ow.T + ob         # (B,1)

Strategy: data-parallel over batch across 8 NeuronCores (8 batches each).
The dominant cost is streaming `distance` (33.5 MB/core) from HBM; 16 hw
DMA queues sustain ~23-26 GB/s each (~400 GB/s aggregate) when fed.
Trace-driven structure:
  * An HWDGE ring holds 4 outstanding transfers and a dma_start on a
    full ring BLOCKS the issuing engine's in-order queue, so steady-
    state distance pushes live on the sync engine only (it does nothing
    else; stalls are harmless).  The scalar ring gets the weight pack +
    batch 0 (3 pushes) before any ACT work exists.
  * Descriptors are 8KB splits of each transfer's contiguous rows,
    round-robined over the 16 queues; 16KB rows made queue 15 a
    2x-loaded straggler.  2MB transfers with max_dma_last_dim=2048
    give 256 even 8KB descriptors AND 8MB of ring-buffered stream.
  * DVE adds run ~1 out/cycle fp32 (~2 packed fp16) but drop ~40% when
    the two operands share an 8KB SBUF bank, and each op has ~200ns
    fixed cost.  So the j-reduction per batch is: two fp32->fp16 pair
    folds (operands 8KB apart), one cross-tile add, one halving ->
    (128,1024); then EIGHT accumulating PE transposes collapse the
    remaining 16 j-partials into a PSUM (128,128) whose rows are
    [even-j sums; odd-j sums](r), copied once to SBUF.  The df matmul
    consumes that directly with a row-stacked Wdf (K=128) - no final
    combine, no separate d_sum transpose.
  * The LAST batch instead streams as four 1MB chunks into padded
    (128,2,2048) tiles (operand halves 8KB apart) with a 5-op fold
    tree per chunk + running combines, so only ~2.5us of fold work
    depends on the final chunk; its df/cf are hoisted and the 3-layer
    chain runs immediately, with other groups' leftovers as filler.
Layer compute runs in batch groups (4,2,1,1): matmuls on PE, bias/tanh
on ACT, elementwise on DVE, software-pipelined so every in-order engine
queue reaches each op with deps already met.  The affine head is folded
on the host into one length-F vector + scalar bias.  All constants ship
in ONE packed array -> one DMA -> one wait sem.
"""

import numpy as np

B, N, F, R, H = 64, 128, 128, 64, 256
L = 3
NCORES = 8
BL = B // NCORES   # batches per core
GROUPS = ((0, 1, 2, 3), (4, 5), (6,), (7,))
TB = BL - 1        # the tail batch (streamed and folded last)

# wpack layout, fp32 columns (fp16 sections hold 2 fp16 per column;
# offsets in the comments below are fp16-element columns of the bitcast view):
#   [0, 384)      wcf lhsT bf16 : bf-col l*H+h           = Wcf_w[l, h, f]
#   [384, 768)    wfc lhsT bf16 : bf-col (l*2+c)*F+f     = Wfc_w[l, f, c*128+hc]
#   [768, 774)    cf bias fp32  : col l*2+c              = Wcf_b[l, c*128+h]
#   [774, 780)    df bias fp32  : col l*2+c              = N * Wdf_b[l, c*128+h]
#   [780, 781)    head lhsT bf16: bf-col 0               = (out_w @ fc0_w)[0, f]
#   [784, 1168)   wdf lhsT bf16 : bf-col l*H+h, row r and row 64+r both
#                                 = Wdf_w[l, h, r]  (stacked for K=128)
#   [1168, 1680)  x bf16        : bf-col b*N+n           = x[b_local, n, f]
#   [1680, 1744)  identity fp16-packed
BCF_OFF = 768
BDF_OFF = 774
HEAD_OFF = 780
HEAD32_OFF = 781
WDF_OFF = 784
XOFF = 1168
IDOFF = 1680
WCOLS = 1808

_CACHE = {}


def _build_program():
    import concourse.bass as bass
    from concourse import bacc
    import concourse.tile as tile
    from concourse import mybir

    f32 = mybir.dt.float32
    bf16 = mybir.dt.float16
    AX = mybir.AxisListType
    AF = mybir.ActivationFunctionType

    nc = bacc.Bacc("TRN2")
    dist = nc.declare_dram_parameter("dist", [BL, N, N, R], f32, isOutput=False)
    wpack = nc.declare_dram_parameter("wpack", [128, WCOLS], f32, isOutput=False)
    out_ext = nc.declare_dram_parameter("out", [BL, 1], f32, isOutput=True)

    with tile.TileContext(nc) as tc:
        with (
            tc.tile_pool(name="consts", bufs=1) as consts,
            tc.tile_pool(name="dist", bufs=4) as dist_pool,
            tc.tile_pool(name="fold", bufs=1) as fold_pool,
            tc.tile_pool(name="dsum", bufs=2) as dsum_pool,
            tc.tile_pool(name="work", bufs=3) as work,
            tc.tile_pool(name="ps1", bufs=1, space="PSUM") as ps1,
            tc.tile_pool(name="ps2", bufs=2, space="PSUM") as ps2,
        ):
            # ---- DMA plumbing -------------------------------------------
            # weight pack: the scalar ring's first push (never stalls ACT)
            wp = consts.tile([128, WCOLS], f32)
            nc.scalar.dma_start(out=wp, in_=wpack[:, :])
            wb = wp.bitcast(bf16)  # (128, 2*WCOLS) bf16 view
            ident = wb[:, 2 * IDOFF : 2 * IDOFF + 128]
            out_acc = consts.tile([1, BL], f32)

            half_tiles = {}   # (b, half) -> 2MB tile, batches 0..TB-1
            tail_tiles = {}   # q -> padded 1MB chunk tile of the tail batch

            def push_half(b, h):
                t = dist_pool.tile([N, 4096], f32, tag="dist", bufs=4)
                dflat = dist[b, :, :, :].rearrange("n j r -> n (j r)")
                eng = nc.scalar if b == 0 else nc.sync
                eng.dma_start(out=t, in_=dflat[:, h * 4096 : (h + 1) * 4096],
                              max_dma_last_dim=2048)
                half_tiles[(b, h)] = t

            def push_tail_chunk(q):
                # halves land 8KB apart so the first fold reads two banks
                t = dist_pool.tile([N, 2, 2048], f32, tag="tail", bufs=4)
                dflat = dist[TB, :, :, :].rearrange("n j r -> n (j r)")
                nc.sync.dma_start(
                    out=t[:, :, 0:1024],
                    in_=dflat[:, q * 2048 : (q + 1) * 2048].rearrange(
                        "n (s w) -> n s w", s=2
                    ),
                )
                tail_tiles[q] = t

            push_queue = [(b, h) for b in range(1, TB) for h in range(2)]
            push_queue += [("t", q) for q in range(4)]
            push_cursor = [0]

            def push_next():
                if push_cursor[0] >= len(push_queue):
                    return
                item = push_queue[push_cursor[0]]
                push_cursor[0] += 1
                if item[0] == "t":
                    push_tail_chunk(item[1])
                else:
                    push_half(*item)

            push_half(0, 0)
            push_half(0, 1)
            for _ in range(2):   # prime sync ring: batch 1
                push_next()       # (4 live tiles total = the tag's bufs)

            def wcf_l(l, c):
                o = l * H + c * 128
                return wb[:, o : o + 128]

            def wdf_l(l, c, kdim):
                o = 2 * WDF_OFF + l * H + c * 128
                return wb[0:kdim, o : o + 128]

            def wfc_l(l, c):
                o = 2 * 384 + (l * 2 + c) * F
                return wb[:, o : o + F]

            def bcf_l(l, c):
                o = BCF_OFF + l * 2 + c
                return wp[:, o : o + 1]

            def bdf_l(l, c):
                o = BDF_OFF + l * 2 + c
                return wp[:, o : o + 1]

            # ---- j-reduction --------------------------------------------
            # batches 0..6: pair folds to (128,1024) = 16 j-partials x 64 r
            folded = {}

            def fold_pair(b):
                t0 = half_tiles.pop((b, 0))
                t1 = half_tiles.pop((b, 1))
                # 8KB-wide tiles so A and B start in different SBUF banks
                a = fold_pool.tile([N, 4096], bf16, tag="fA", name="fA")
                nc.vector.tensor_add(a[:, 0:2048], t0[:, 0:2048], t0[:, 2048:4096])
                push_next()
                bt = fold_pool.tile([N, 2048], bf16, tag="fB", name="fB")
                nc.vector.tensor_add(bt, t1[:, 0:2048], t1[:, 2048:4096])
                push_next()
                c = fold_pool.tile([N, 2048], bf16, tag="fC", name="fC")
                nc.vector.tensor_add(c, a[:, 0:2048], bt)
                d = dsum_pool.tile([N, 1024], bf16, tag="fD", name=f"fD{b}",
                                   bufs=4)
                nc.vector.tensor_add(d, c[:, 0:1024], c[:, 1024:2048])
                folded[b] = d

            def emit_tp(b, dsT, k):
                # 8 accumulating PE transposes: psum rows 0:64 = even-j'
                # partial sums over r, rows 64:128 = odd-j'; the stacked
                # Wdf contracts both halves in one K=128 matmul.
                d = folded.pop(b)
                p = ps1.tile([128, N], f32, tag="P", name=f"P{b}")
                for s in range(8):
                    # REAL matmul against identity: out[m,q] += d[q, 128s+m]
                    # (is_transpose mode would overwrite instead of accumulate)
                    nc.tensor.matmul(p, d[:, 128 * s : 128 * (s + 1)], ident,
                                     start=(s == 0), stop=(s == 7))
                nc.scalar.activation(out=dsT[:, k * N : (k + 1) * N], in_=p,
                                     func=AF.Copy)

            # tail batch: per-chunk 5-op fold trees + running combines
            tstate = {}

            def fold_tail_chunk(q):
                src = tail_tiles.pop(q)
                f1 = fold_pool.tile([N, 1024], bf16, tag="t1", name="t1")
                nc.vector.tensor_add(f1, src[:, 0, 0:1024], src[:, 1, 0:1024])
                u = fold_pool.tile([N, 512], bf16, tag="t2", name="t2")
                nc.vector.tensor_add(u, f1[:, 0:512], f1[:, 512:1024])
                v = fold_pool.tile([N, 256], bf16, tag="t3", name="t3")
                nc.vector.tensor_add(v, u[:, 0:256], u[:, 256:512])
                nc.vector.tensor_add(u[:, 0:128], v[:, 0:128], v[:, 128:256])
                qt = dsum_pool.tile([N, R], bf16, tag="tq", name=f"tq{q}", bufs=4)
                nc.vector.tensor_add(qt, u[:, 0:64], u[:, 64:128])
                if q == 0:
                    tstate["acc"] = qt
                else:
                    r = dsum_pool.tile([N, R], bf16, tag="tacc",
                                       name=f"tacc{q}", bufs=2)
                    nc.vector.tensor_add(r, tstate["acc"], qt)
                    tstate["acc"] = r

            # ---- layer pipeline (per batch group) -----------------------
            gstate = {}

            def emit_group_tp(gi):
                bs = GROUPS[gi]
                dsT = dsum_pool.tile([128, 4 * N], bf16, tag="dsT",
                                     name=f"dsT{gi}")
                for k, b in enumerate(bs):
                    emit_tp(b, dsT, k)
                NG = len(bs) * N
                xc = wb[:, 2 * XOFF + bs[0] * N : 2 * XOFF + (bs[-1] + 1) * N]
                gstate.setdefault(gi, {}).update(
                    {"dsT": dsT, "xc": xc, "NG": NG, "bs": bs, "kdim": 128}
                )

            def emit_tail_trs(gi):
                # classic path for the tail batch: dsum (128,64) -> (64,128)
                trp = ps1.tile([128, N], f32, tag="P",
                               name="tr").bitcast(bf16)[0:R, 0:N]
                nc.tensor.transpose(trp, tstate["acc"], ident)
                dsT = dsum_pool.tile([R, N], bf16, tag="dsT7", name="dsT7")
                nc.scalar.activation(out=dsT, in_=trp, func=AF.Copy)
                bs = GROUPS[gi]
                xc = wb[:, 2 * XOFF + bs[0] * N : 2 * XOFF + (bs[-1] + 1) * N]
                gstate.setdefault(gi, {}).update(
                    {"dsT": dsT, "xc": xc, "NG": N, "bs": bs, "kdim": R}
                )

            def emit_cf_hoist(gi):
                # layer-0 cf depends only on the x pack: compute any time
                bs = GROUPS[gi]
                NG = len(bs) * N
                xc = wb[:, 2 * XOFF + bs[0] * N : 2 * XOFF + (bs[-1] + 1) * N]
                res = []
                for c in range(2):
                    cfp = ps1.tile([128, 4 * N], f32, tag=f"cf{c}",
                                   name=f"cfp{c}")[:, 0:NG]
                    nc.tensor.matmul(cfp, wcf_l(0, c), xc, start=True, stop=True)
                    cfs = work.tile([128, N], bf16, tag=f"cfH{gi}{c}",
                                    name=f"cfH{gi}{c}", bufs=1)[:, 0:NG]
                    nc.scalar.activation(out=cfs, in_=cfp, func=AF.Identity,
                                         bias=bcf_l(0, c))
                    res.append(cfs)
                gstate.setdefault(gi, {})["cfs0"] = res

            def emit_cf(gi, l):
                st = gstate[gi]
                NG, xc = st["NG"], st["xc"]
                res = []
                for c in range(2):
                    cfp = ps1.tile([128, 4 * N], f32, tag=f"cf{c}",
                                   name=f"cfp{c}")[:, 0:NG]
                    nc.tensor.matmul(cfp, wcf_l(l, c), xc, start=True, stop=True)
                    cfs = work.tile([128, 4 * N], bf16, tag=f"cfs{c}",
                                    name=f"cfs{c}")[:, 0:NG]
                    nc.scalar.activation(out=cfs, in_=cfp, func=AF.Identity,
                                         bias=bcf_l(l, c))
                    res.append(cfs)
                return res

            def emit_df_hoist(gi):
                # df for every layer depends only on dsT: compute it all
                # right after the tail transpose (shrinks the tail group's
                # post-fold critical chain to cf/mul/fc/tanh/add only)
                st = gstate[gi]
                NG, dsT, kdim = st["NG"], st["dsT"], st["kdim"]
                st["dfs"] = {}
                for l in range(L):
                    for c in range(2):
                        dfp = ps1.tile([128, 4 * N], f32, tag=f"df{c}",
                                       name=f"dfp{c}")[:, 0:NG]
                        nc.tensor.matmul(dfp, wdf_l(l, c, kdim), dsT[:, 0:NG],
                                         start=True, stop=True)
                        dfs = work.tile([128, N], bf16, tag=f"dfsP{c}{l}",
                                        name=f"dfsP{c}{l}", bufs=1)[:, 0:NG]
                        nc.scalar.activation(out=dfs, in_=dfp, func=AF.Identity,
                                             bias=bdf_l(l, c))
                        st["dfs"][(l, c)] = dfs

            def emit_layer(gi, l):
                st = gstate[gi]
                NG, dsT, kdim = st["NG"], st["dsT"], st["kdim"]
                pre_dfs = st.get("dfs")
                pre_cfs = st.pop("cfs0", None) if l == 0 else None
                if pre_cfs is None:
                    pre_cfs = emit_cf(gi, l)
                ms = []
                for c in range(2):
                    cfs = pre_cfs[c]
                    if pre_dfs is not None:
                        dfs = pre_dfs.pop((l, c))
                    else:
                        dfp = ps1.tile([128, 4 * N], f32, tag=f"df{c}",
                                       name=f"dfp{c}")[:, 0:NG]
                        nc.tensor.matmul(dfp, wdf_l(l, c, kdim), dsT[:, 0:NG],
                                         start=True, stop=True)
                        dfs = work.tile([128, 4 * N], bf16, tag=f"dfs{c}",
                                        name=f"dfs{c}")[:, 0:NG]
                        nc.scalar.activation(out=dfs, in_=dfp, func=AF.Identity,
                                             bias=bdf_l(l, c))
                    m = work.tile([128, 4 * N], bf16, tag=f"m{c}",
                                  name=f"m{c}", bufs=2)[:, 0:NG]
                    nc.vector.tensor_mul(m, cfs, dfs)
                    ms.append(m)
                hp = ps2.tile([F, 4 * N], f32, tag="h", name="hp")[:, 0:NG]
                nc.tensor.matmul(hp, wfc_l(l, 0), ms[0], start=True, stop=False)
                nc.tensor.matmul(hp, wfc_l(l, 1), ms[1], start=False, stop=True)
                th = work.tile([F, 4 * N], f32, tag="t", name="th",
                               bufs=2)[:, 0:NG]
                nc.scalar.activation(out=th, in_=hp, func=AF.Tanh)
                xdt = f32 if l == L - 1 else bf16
                xn = work.tile([F, 4 * N], xdt, tag=f"x{l}", name="xn")[:, 0:NG]
                nc.vector.tensor_add(xn, hp, th)
                st["xc"] = xn

            def emit_head(gi):
                st = gstate[gi]
                NG, bs = st["NG"], st["bs"]
                G = len(bs)
                hd = ps1.tile([128, 4 * N], f32, tag="cf0", name="hd")[0:1, 0:NG]
                nc.tensor.matmul(hd, wp[:, HEAD32_OFF : HEAD32_OFF + 1],
                                 st["xc"], start=True, stop=True)
                nc.vector.tensor_reduce(
                    out=out_acc[0:1, bs[0] : bs[0] + G],
                    in_=hd.rearrange("o (b n) -> o b n", b=G),
                    axis=AX.X,
                    op=mybir.AluOpType.add,
                )

            # ---- software-pipelined schedule ----------------------------
            # In-order engine queues: folds run as data lands (never behind
            # layer ops whose deps aren't ready); the tail group's chain
            # starts immediately after the last fold with earlier groups'
            # leftovers as latency-hiding filler.
            for b in (0, 1, 2, 3):
                fold_pair(b)
            emit_group_tp(0)
            emit_cf_hoist(2)
            emit_cf_hoist(3)
            fold_pair(4)
            emit_layer(0, 0)
            fold_pair(5)
            emit_layer(0, 1)
            emit_group_tp(1)
            emit_layer(0, 2)
            emit_head(0)
            fold_pair(6)
            emit_layer(1, 0)
            emit_group_tp(2)
            emit_layer(1, 1)
            fold_tail_chunk(0)
            fold_tail_chunk(1)
            fold_tail_chunk(2)
            emit_layer(2, 0)
            fold_tail_chunk(3)
            emit_layer(1, 2)
            emit_head(1)
            emit_tail_trs(3)
            emit_df_hoist(3)
            emit_layer(3, 0)
            emit_layer(2, 1)
            emit_layer(3, 1)
            emit_layer(2, 2)
            emit_head(2)
            emit_layer(3, 2)
            emit_head(3)

            nc.sync.dma_start(out=out_ext.rearrange("b o -> o b"), in_=out_acc)

    return nc


def _host_pack(x, Wcf_w, Wcf_b, Wdf_w, Wdf_b, Wfc_w, fc0_w, fc0_b, out_w, out_b):
    import ml_dtypes

    f = np.float32
    bf = np.float16

    def pack_bf(a):  # (128, 2K) bf16 -> (128, K) fp32 bit-packed
        return np.ascontiguousarray(a.astype(bf)).view(f)

    base = np.zeros((128, WCOLS), f)
    base[:, 0:384] = pack_bf(np.asarray(Wcf_w, f).transpose(2, 0, 1).reshape(128, L * H))
    base[:, 384:768] = pack_bf(
        np.asarray(Wfc_w, f).reshape(L, F, 2, 128).transpose(3, 0, 2, 1).reshape(128, L * 2 * F)
    )
    base[:, BCF_OFF : BCF_OFF + 6] = (
        np.asarray(Wcf_b, f).reshape(L, 2, 128).transpose(2, 0, 1).reshape(128, 6)
    )
    base[:, BDF_OFF : BDF_OFF + 6] = (
        (N * np.asarray(Wdf_b, f)).reshape(L, 2, 128).transpose(2, 0, 1).reshape(128, 6)
    )
    w_head = (np.asarray(out_w, np.float64) @ np.asarray(fc0_w, np.float64))[0]  # (F,)
    head_pair = np.zeros((128, 2), f)
    head_pair[:, 0] = w_head.astype(f)
    base[:, HEAD_OFF : HEAD_OFF + 1] = pack_bf(head_pair)
    base[:, HEAD32_OFF] = w_head.astype(f)
    wdf_cols = pack_bf(np.asarray(Wdf_w, f).transpose(2, 0, 1).reshape(R, L * H))
    base[0:R, WDF_OFF : WDF_OFF + 384] = wdf_cols
    base[R : 2 * R, WDF_OFF : WDF_OFF + 384] = wdf_cols  # stacked for K=128
    base[:, IDOFF : IDOFF + 64] = pack_bf(np.eye(128, dtype=f))

    b_head = float((np.asarray(out_w, np.float64) @ np.asarray(fc0_b, np.float64)
                    + np.asarray(out_b, np.float64)).reshape(()))

    x_t = np.asarray(x, f).transpose(0, 2, 1)  # (B, F, N)
    wpacks = []
    for i in range(NCORES):
        wp = base.copy()
        wp[:, XOFF : XOFF + BL * N // 2] = pack_bf(
            x_t[i * BL : (i + 1) * BL].transpose(1, 0, 2).reshape(128, BL * N)
        )
        wpacks.append(wp)
    return wpacks, b_head


def run(trace=False, **inputs):
    from concourse.bass_utils import run_bass_kernel_spmd

    distance = np.ascontiguousarray(np.asarray(inputs["distance"], np.float32))
    wpacks, b_head = _host_pack(
        inputs["x"], inputs["Wcf_w"], inputs["Wcf_b"], inputs["Wdf_w"], inputs["Wdf_b"],
        inputs["Wfc_w"], inputs["fc0_w"], inputs["fc0_b"], inputs["out_w"], inputs["out_b"],
    )

    if "nc" not in _CACHE:
        nc = _build_program()
        nc.finalize()
        _CACHE["nc"] = nc
    nc = _CACHE["nc"]

    in_maps = []
    for i in range(NCORES):
        in_maps.append({
            "dist": np.ascontiguousarray(distance[i * BL : (i + 1) * BL]),
            "wpack": wpacks[i],
        })
    res = run_bass_kernel_spmd(nc, in_maps, list(range(NCORES)), trace=trace)
    out = np.concatenate([res.results[i]["out"] for i in range(NCORES)], axis=0)
    out = (out.astype(np.float64) + b_head).astype(np.float32)
    return out, res


def kernel(**inputs):
    out, _ = run(trace=False, **inputs)
    return out
